# revision 1
# baseline (speedup 1.0000x reference)
"""3-layer GAT on Trainium2 — 8-core SPMD Bass kernel.

Node-partitioned (edge-cut) distribution:
- core c owns nodes [c*NPC, (c+1)*NPC) and all edges whose dst lands there
- per layer: each core computes its nodes' h_ext = in @ [W | W@As | W@Ad],
  publishes rows [h | als | ald | pad] to a replicated table via AllGather,
  then processes its incoming edges in windows of 128 dst nodes:
  dma_gather of source rows, exp(leaky(als+ald)) on-chip, scatter-add via
  one-hot matmul into PSUM, normalize by the per-node sum at the end.
"""
from dataclasses import dataclass, field

import numpy as np

import concourse.bass as bass
import concourse.bacc as bacc
import concourse.mybir as mybir
import concourse.tile as tile
from concourse.masks import make_identity

P = 128
N_CORES = 8
NEG_SLOPE = 0.2
DUMMY_ALS = -1000.0
ALD_PAY = 64  # floats gathered for the dst-side (als|ald|pad) payload


@dataclass
class LayerCfg:
    fin: int
    fout: int
    H: int
    C: int
    drow: int      # table row width in f32 (multiple of 64)
    relu: bool

    @property
    def ex(self):
        return 2 * self.H

    @property
    def fw(self):
        return self.fout + self.ex


@dataclass
class GatCfg:
    n: int
    n_cores: int = N_CORES
    layers: tuple = ()
    # filled by preprocessing
    t_a: int = 0
    t_b: int = 0

    @property
    def npc(self):
        return self.n // self.n_cores

    @property
    def n_win(self):
        return (self.npc + P - 1) // P

    @property
    def rows(self):
        return self.npc + 1

    @property
    def half(self):
        return self.rows * (self.n_cores // 2)

    @property
    def half_nodes(self):
        return self.npc * (self.n_cores // 2)

    @property
    def tbl(self):
        return self.rows * self.n_cores

    @property
    def t(self):
        return self.t_a + self.t_b


def real_cfg():
    return GatCfg(
        n=50000,
        layers=(
            LayerCfg(256, 256, 8, 32, 320, True),
            LayerCfg(256, 256, 8, 32, 320, True),
            LayerCfg(256, 64, 1, 64, 128, False),
        ),
    )


def mini_cfg():
    return GatCfg(
        n=1024,
        layers=(
            LayerCfg(256, 256, 8, 32, 320, True),
            LayerCfg(256, 256, 8, 32, 320, True),
            LayerCfg(256, 64, 1, 64, 128, False),
        ),
    )


# ---------------------------------------------------------------- host prep

def _wrap16(idx_list):
    """dma_gather index layout: idx i lives at [i % 16, i // 16]; replicate
    the 16-partition block 8x down to 128 partitions."""
    a = idx_list.reshape(-1, 16).T  # [16, len/16]
    return np.tile(a, (8, 1)).astype(np.int16)


def preprocess(cfg: GatCfg, edge_index):
    """Partition edges by dst core, group into 128-dst-node windows, split by
    src half (int16 index range), pad to uniform tile counts.

    Returns (per_core_index_inputs, cfg with t_a/t_b set).
    """
    npc, n_win = cfg.npc, cfg.n_win
    src = np.concatenate([edge_index[0], np.arange(cfg.n, dtype=np.int64)])
    dst = np.concatenate([edge_index[1], np.arange(cfg.n, dtype=np.int64)])

    core = dst // npc
    # storage index of a node inside the 8*(npc+1)-row gathered table
    sidx = (src // npc) * cfg.rows + (src % npc)
    is_b = sidx >= cfg.half

    per_core = []
    counts_a = np.zeros((cfg.n_cores, n_win), dtype=np.int64)
    counts_b = np.zeros((cfg.n_cores, n_win), dtype=np.int64)
    for c in range(cfg.n_cores):
        sel = np.nonzero(core == c)[0]
        d_loc = (dst[sel] - c * npc).astype(np.int64)
        w = d_loc // P
        half_flag = is_b[sel].astype(np.int64)
        order = np.lexsort((d_loc, half_flag, w))
        sel = sel[order]
        d_loc = d_loc[order]
        w = w[order]
        half_flag = half_flag[order]
        rel = (sidx[sel] - half_flag * cfg.half).astype(np.int64)
        for wi in range(n_win):
            m = w == wi
            counts_a[c, wi] = int((m & (half_flag == 0)).sum())
            counts_b[c, wi] = int((m & (half_flag == 1)).sum())
        per_core.append((d_loc, w, half_flag, rel))

    t_a = int(np.ceil(counts_a.max() / P))
    t_b = int(np.ceil(counts_b.max() / P))
    t_a = max(t_a, 1)
    t_b = max(t_b, 1)
    cfg.t_a, cfg.t_b = t_a, t_b
    t = t_a + t_b
    dummy = cfg.npc  # dummy row index, valid in A-rel, B-rel and local coords

    idx_inputs = []
    for c in range(cfg.n_cores):
        d_loc, w, half_flag, rel = per_core[c]
        idx16 = np.zeros((n_win, P, 2 * t * 8), dtype=np.int16)
        dst32 = np.zeros((n_win, P, t), dtype=np.int32)
        for wi in range(n_win):
            m = w == wi
            ra = rel[m & (half_flag == 0)]
            rb = rel[m & (half_flag == 1)]
            da = d_loc[m & (half_flag == 0)] % P
            db = d_loc[m & (half_flag == 1)] % P
            src_list = np.full(t * P, dummy, dtype=np.int64)
            ald_list = np.full(t * P, dummy, dtype=np.int64)
            dl_list = np.zeros(t * P, dtype=np.int64)
            src_list[: len(ra)] = ra
            src_list[t_a * P : t_a * P + len(rb)] = rb
            # ald gather reads the core-local table -> local dst index
            ald_list[: len(da)] = da + (wi * P)
            ald_list[t_a * P : t_a * P + len(db)] = db + (wi * P)
            dl_list[: len(da)] = da
            dl_list[t_a * P : t_a * P + len(db)] = db
            wa = _wrap16(src_list[: t_a * P])
            wb = _wrap16(src_list[t_a * P :])
            wl = _wrap16(ald_list)
            idx16[wi] = np.concatenate([wa, wb, wl], axis=1)
            # dstloc in (p, j) layout: edge i -> [i % 128, i // 128]
            dst32[wi] = dl_list.reshape(t, P).T.astype(np.int32)
        idx_inputs.append({"idx16": idx16, "dst32": dst32})
    return idx_inputs


def shard_inputs(cfg: GatCfg, inputs):
    """Build the per-core input dicts for run_bass_kernel_spmd."""
    x = np.asarray(inputs["x"], dtype=np.float32)
    edge_index = np.asarray(inputs["edge_index"])
    idx_inputs = preprocess(cfg, edge_index)

    def blockdiag(a_s, a_d, fin_rows):
        H, C = a_s.shape
        A = np.zeros((fin_rows, 2 * H), dtype=np.float32)
        for h in range(H):
            A[h * C : (h + 1) * C, h] = a_s[h]
            A[h * C : (h + 1) * C, H + h] = a_d[h]
        return A

    weight_common = {}
    for li, nm in enumerate(["1", "2", "3"]):
        lc = cfg.layers[li]
        Wn = np.asarray(inputs[f"W{nm}"], dtype=np.float32)
        weight_common[f"W{nm}"] = np.ascontiguousarray(Wn)
        weight_common[f"Wt{nm}"] = np.ascontiguousarray(Wn.T)
        weight_common[f"A{nm}"] = blockdiag(
            np.asarray(inputs[f"as{nm}"], np.float32),
            np.asarray(inputs[f"ad{nm}"], np.float32),
            lc.fout,
        )
        weight_common[f"b{nm}"] = np.asarray(inputs[f"b{nm}"], np.float32).reshape(1, -1)

    in_maps = []
    for c in range(cfg.n_cores):
        xs = np.zeros((cfg.n_win * P, cfg.layers[0].fin), dtype=np.float32)
        xs[: cfg.npc] = x[c * cfg.npc : (c + 1) * cfg.npc]
        m = {"x_sh": xs, **idx_inputs[c], **weight_common}
        in_maps.append(m)
    return in_maps


# ---------------------------------------------------------------- device code

def build_program(cfg: GatCfg, repeats: int = 1):
    nc = bacc.Bacc("TRN2", target_bir_lowering=False, debug=False,
                   num_devices=cfg.n_cores)
    n_win, t_a, t_b, t = cfg.n_win, cfg.t_a, cfg.t_b, cfg.t
    f32 = mybir.dt.float32

    # ---- I/O
    x_sh = nc.dram_tensor("x_sh", [n_win * P, cfg.layers[0].fin], f32,
                          kind="ExternalInput")
    idx16 = nc.dram_tensor("idx16", [n_win, P, 2 * t * 8], mybir.dt.int16,
                           kind="ExternalInput")
    dst32 = nc.dram_tensor("dst32", [n_win, P, t], mybir.dt.int32,
                           kind="ExternalInput")
    wt_in = {}
    for li, nm in enumerate(["1", "2", "3"]):
        lc = cfg.layers[li]
        wt_in[f"W{nm}"] = nc.dram_tensor(f"W{nm}", [lc.fin, lc.fout], f32,
                                         kind="ExternalInput")
        wt_in[f"Wt{nm}"] = nc.dram_tensor(f"Wt{nm}", [lc.fout, lc.fin], f32,
                                          kind="ExternalInput")
        wt_in[f"A{nm}"] = nc.dram_tensor(f"A{nm}", [lc.fout, lc.ex], f32,
                                         kind="ExternalInput")
        wt_in[f"b{nm}"] = nc.dram_tensor(f"b{nm}", [1, lc.fout], f32,
                                         kind="ExternalInput")
    out_t = nc.dram_tensor("out", [cfg.npc, cfg.layers[-1].fout], f32,
                           kind="ExternalOutput")

    # ---- internal DRAM
    hal12_loc = nc.dram_tensor("hal12_loc", [cfg.rows, 320], f32, kind="Internal")
    hal12_full = nc.dram_tensor("hal12_full", [cfg.tbl, 320], f32,
                                kind="Internal", addr_space="Shared")
    hal3_loc = nc.dram_tensor("hal3_loc", [cfg.rows, 128], f32, kind="Internal")
    hal3_full = nc.dram_tensor("hal3_full", [cfg.tbl, 128], f32,
                               kind="Internal", addr_space="Shared")

    iota_const = nc.inline_tensor(
        np.tile(np.arange(P, dtype=np.float32), (P, 1)), name="iota_const")

    rg = [list(range(cfg.n_cores))]

    with tile.TileContext(nc) as tc:
        import contextlib
        with contextlib.ExitStack() as ctx:
            persist = ctx.enter_context(tc.tile_pool(name="persist", bufs=1))
            wpool = ctx.enter_context(tc.tile_pool(name="wts", bufs=1))
            sb = ctx.enter_context(tc.tile_pool(name="work", bufs=2))
            gp = ctx.enter_context(tc.tile_pool(name="gathp", bufs=3))
            sm = ctx.enter_context(tc.tile_pool(name="small", bufs=4))
            fr = ctx.enter_context(tc.tile_pool(name="front", bufs=2))
            ps = ctx.enter_context(tc.tile_pool(name="psum", bufs=2, space="PSUM"))
            psf = ctx.enter_context(tc.tile_pool(name="psumf", bufs=2, space="PSUM"))

            identity = persist.tile([P, P], f32)
            make_identity(nc, identity[:])
            iota_sb = persist.tile([P, P], f32)
            nc.sync.dma_start(out=iota_sb[:], in_=iota_const[:])

            in_local = persist.tile([P, n_win, cfg.layers[0].fin], f32)
            for _rep in range(repeats):
              for w in range(n_win):
                nc.sync.dma_start(out=in_local[:, w, :],
                                  in_=x_sh[w * P : (w + 1) * P, :])

              for li in range(len(cfg.layers)):
                  lc = cfg.layers[li]
                  nm = str(li + 1)
                  hal_loc = hal12_loc if lc.drow == 320 else hal3_loc
                  hal_full = hal12_full if lc.drow == 320 else hal3_full
                  kch = lc.fin // P  # input chunks (contraction)

                  # ---------- per-layer constants
                  w_ext = wpool.tile([P, kch, 272], f32, tag="w_ext")
                  nc.sync.dma_start(
                      out=w_ext[:, :, : lc.fout],
                      in_=wt_in[f"W{nm}"][:].rearrange("(q p) f -> p q f", p=P))
                  # Wt rows = fout (contraction dim of W @ A)
                  kout = (lc.fout + P - 1) // P
                  kpart = min(lc.fout, P)
                  wt_sb = wpool.tile([kpart, kout, lc.fin], f32, tag="wt_sb")
                  nc.sync.dma_start(
                      out=wt_sb[:, :, :],
                      in_=wt_in[f"Wt{nm}"][:].rearrange("(q p) f -> p q f", p=kpart))
                  a_sb = wpool.tile([kpart, kout, lc.ex], f32, tag="a_sb")
                  nc.sync.dma_start(
                      out=a_sb[:, :, :],
                      in_=wt_in[f"A{nm}"][:].rearrange("(q p) e -> p q e", p=kpart))
                  for mq in range(kch):
                      wa_ps = psf.tile([P, lc.ex], f32, tag="wa_ps")
                      for q in range(kout):
                          nc.tensor.matmul(
                              wa_ps[:],
                              lhsT=wt_sb[:, q, mq * P : (mq + 1) * P],
                              rhs=a_sb[:, q, :],
                              start=(q == 0), stop=(q == kout - 1))
                      nc.vector.tensor_copy(
                          out=w_ext[:, mq, lc.fout : lc.fw], in_=wa_ps[:])
                  ald_all = wpool.tile([P, n_win * lc.H], f32, tag="ald_all")
                  b_t = sm.tile([1, lc.fout], f32, tag="b_t")
                  nc.sync.dma_start(out=b_t[:], in_=wt_in[f"b{nm}"][:])
                  b_bc = wpool.tile([P, lc.fout], f32, tag="b_bc")
                  nc.gpsimd.partition_broadcast(b_bc[:], b_t[:1, :])

                  # ---------- front phase: h_ext for own nodes -> hal_loc
                  for w in range(n_win):
                      in_t = fr.tile([P, lc.fin], f32, tag="in_t")
                      for q in range(kch):
                          tp = psf.tile([P, P], f32, tag="tp")
                          nc.tensor.transpose(
                              tp[:], in_local[:, w, q * P : (q + 1) * P],
                              identity[:])
                          nc.vector.tensor_copy(
                              out=in_t[:, q * P : (q + 1) * P], in_=tp[:])
                      h_ps = psf.tile([P, lc.fw], f32, tag="h_ps")
                      for q in range(kch):
                          nc.tensor.matmul(
                              h_ps[:],
                              lhsT=in_t[:, q * P : (q + 1) * P],
                              rhs=w_ext[:, q, : lc.fw],
                              start=(q == 0), stop=(q == kch - 1))
                      nc.scalar.copy(
                          out=ald_all[:, w * lc.H : (w + 1) * lc.H],
                          in_=h_ps[:, lc.fout + lc.H : lc.fout + 2 * lc.H])
                      stage = fr.tile([P, lc.drow], f32, tag="stage")
                      nc.vector.tensor_copy(out=stage[:, : lc.fw], in_=h_ps[:])
                      if lc.fw < lc.drow:
                          nc.vector.memset(stage[:, lc.fw :], 0.0)
                      rows = min(P, cfg.npc - w * P)
                      nc.sync.dma_start(
                          out=hal_loc[w * P : w * P + rows, :],
                          in_=stage[:rows, :])
                  # dummy row
                  dmy = sm.tile([1, lc.drow], f32, tag="dmy")
                  nc.vector.memset(dmy[:], 0.0)
                  nc.vector.memset(dmy[:1, lc.fout : lc.fout + lc.H], DUMMY_ALS)
                  nc.sync.dma_start(out=hal_loc[cfg.npc : cfg.npc + 1, :],
                                    in_=dmy[:])

                  nc.gpsimd.collective_compute(
                      "AllGather", mybir.AluOpType.bypass,
                      ins=[hal_loc[:]], outs=[hal_full[:]],
                      replica_groups=rg)

                  # ---------- edge phase
                  for w in range(n_win):
                      idx_t = sm.tile([P, t * 8], mybir.dt.int16, tag="idx_t")
                      nc.scalar.dma_start(out=idx_t[:], in_=idx16[w, :, : t * 8])
                      dl_t = sm.tile([P, t], mybir.dt.int32, tag="dl_t")
                      nc.scalar.dma_start(out=dl_t[:], in_=dst32[w, :, :])

                      gath = gp.tile([P, t, lc.drow], f32, tag="gath")
                      nc.gpsimd.dma_gather(
                          gath[:, :t_a, :], hal_full[: cfg.half, :],
                          idx_t[:, : t_a * 8],
                          num_idxs=t_a * P, num_idxs_reg=t_a * P,
                          elem_size=lc.drow, single_packet=False)
                      nc.gpsimd.dma_gather(
                          gath[:, t_a:, :], hal_full[cfg.half :, :],
                          idx_t[:, t_a * 8 : t * 8],
                          num_idxs=t_b * P, num_idxs_reg=t_b * P,
                          elem_size=lc.drow, single_packet=False)
                      # one-hot M for the whole window (needed early: ald bcast)
                      dl_f = sm.tile([P, t], f32, tag="dl_f")
                      nc.vector.tensor_copy(out=dl_f[:], in_=dl_t[:])
                      m_all = sb.tile([P, t * P], f32, tag="m_all")
                      nc.vector.tensor_tensor(
                          out=m_all[:].rearrange("p (t n) -> p t n", t=t),
                          in0=iota_sb[:].unsqueeze(1).to_broadcast([P, t, P]),
                          in1=dl_f[:].unsqueeze(2).to_broadcast([P, t, P]),
                          op=mybir.AluOpType.is_equal)

                      # ald[dst] per edge via PE: Mt_j @ ald_win
                      aldps = psf.tile([P, t * lc.H], f32, tag="wa_ps")
                      for j in range(t):
                          mt_ps = psf.tile([P, P], f32, tag="tp")
                          nc.tensor.transpose(
                              mt_ps[:], m_all[:, j * P : (j + 1) * P],
                              identity[:])
                          mt_sb = sb.tile([P, P], f32, tag="mt_sb")
                          nc.scalar.copy(out=mt_sb[:], in_=mt_ps[:])
                          nc.tensor.matmul(
                              aldps[:, j * lc.H : (j + 1) * lc.H],
                              lhsT=mt_sb[:],
                              rhs=ald_all[:, w * lc.H : (w + 1) * lc.H],
                              start=True, stop=True)

                      # e0 = als[src] + ald[dst]
                      e0 = sm.tile([P, t * lc.H], f32, tag="e0")
                      nc.vector.tensor_tensor(
                          out=e0[:].rearrange("p (t h) -> p t h", t=t),
                          in0=gath[:, :, lc.fout : lc.fout + lc.H],
                          in1=aldps[:].rearrange("p (t h) -> p t h", t=t),
                          op=mybir.AluOpType.add)
                      # exp(leaky(e0)) = Exp(0.6 * (e0 + (2/3)|e0|))
                      t1 = sm.tile([P, t * lc.H], f32, tag="t1")
                      nc.scalar.activation(t1[:], e0[:],
                                           mybir.ActivationFunctionType.Abs,
                                           scale=2.0 / 3.0)
                      nc.vector.tensor_tensor(out=e0[:], in0=e0[:], in1=t1[:],
                                              op=mybir.AluOpType.add)
                      nc.scalar.activation(
                          gath[:, :, lc.fout : lc.fout + lc.H],
                          e0[:].rearrange("p (t h) -> p t h", t=t),
                          mybir.ActivationFunctionType.Exp, scale=0.6)

                      # msg scale: h *= exp_e (broadcast over C)
                      h_view = gath[:, :, : lc.fout].rearrange(
                          "p t (h c) -> p t h c", h=lc.H)
                      expv = gath[:, :, lc.fout : lc.fout + lc.H].unsqueeze(
                          3).to_broadcast([P, t, lc.H, lc.C])
                      nc.vector.tensor_tensor(out=h_view, in0=h_view, in1=expv,
                                              op=mybir.AluOpType.mult)

                      # scatter-add via PE
                      acc = ps.tile([P, lc.fout + lc.H], f32, tag="acc")
                      for j in range(t):
                          nc.tensor.matmul(
                              acc[:],
                              lhsT=m_all[:, j * P : (j + 1) * P],
                              rhs=gath[:, j, : lc.fout + lc.H],
                              start=(j == 0), stop=(j == t - 1))

                      # normalize + bias (+relu) at node level
                      sden = sm.tile([P, lc.H], f32, tag="sden")
                      nc.vector.tensor_scalar(
                          out=sden[:], in0=acc[:, lc.fout : lc.fout + lc.H],
                          scalar1=1e-12, scalar2=None, op0=mybir.AluOpType.max)
                      rec = sm.tile([P, lc.H], f32, tag="rec")
                      nc.vector.reciprocal(rec[:], sden[:])
                      recv = rec[:].unsqueeze(2).to_broadcast([P, lc.H, lc.C])
                      o_t = fr.tile([P, lc.fout], f32, tag="o_t")
                      nc.vector.tensor_tensor(
                          out=o_t[:].rearrange("p (h c) -> p h c", h=lc.H),
                          in0=acc[:, : lc.fout].rearrange(
                              "p (h c) -> p h c", h=lc.H),
                          in1=recv, op=mybir.AluOpType.mult)
                      nc.vector.tensor_tensor(out=o_t[:], in0=o_t[:], in1=b_bc[:],
                                              op=mybir.AluOpType.add)
                      if lc.relu:
                          nc.vector.tensor_scalar(
                              out=in_local[:, w, : lc.fout], in0=o_t[:],
                              scalar1=0.0, scalar2=None, op0=mybir.AluOpType.max)
                      else:
                          rows = min(P, cfg.npc - w * P)
                          nc.sync.dma_start(
                              out=out_t[w * P : w * P + rows, :],
                              in_=o_t[:rows, :])

    nc.compile()
    return nc


# ---------------------------------------------------------------- runner

def _make_pjrt_fn(nc, n_cores):
    """Cached PJRT executable for nc (modeled on bass2jax.run_bass_via_pjrt,
    without output-buffer donation so it can be re-invoked for timing)."""
    import jax
    from jax.sharding import Mesh, PartitionSpec
    from jax.experimental.shard_map import shard_map
    from concourse import bass2jax, mybir as mb

    bass2jax.install_neuronx_cc_hook()
    partition_name = nc.partition_id_tensor.name if nc.partition_id_tensor else None
    in_names, out_names, out_avals, zero_outs = [], [], [], []
    for alloc in nc.m.functions[0].allocations:
        if not isinstance(alloc, mb.MemoryLocationSet):
            continue
        name = alloc.memorylocations[0].name
        if alloc.kind == "ExternalInput":
            if name != partition_name:
                in_names.append(name)
        elif alloc.kind == "ExternalOutput":
            out_names.append(name)
            shape = tuple(alloc.tensor_shape)
            dtype = mb.dt.np(alloc.dtype)
            out_avals.append(jax.core.ShapedArray(shape, dtype))
            zero_outs.append(np.zeros(shape, dtype))
    n_params = len(in_names)
    all_in_names = list(in_names) + list(out_names)
    if partition_name is not None:
        all_in_names.append(partition_name)

    def _body(*args):
        operands = list(args)
        if partition_name is not None:
            operands.append(bass2jax.partition_id_tensor())
        outs = bass2jax._bass_exec_p.bind(
            *operands,
            out_avals=tuple(out_avals),
            in_names=tuple(all_in_names),
            out_names=tuple(out_names),
            lowering_input_output_aliases=(),
            sim_require_finite=True,
            sim_require_nnan=True,
            nc=nc,
        )
        return tuple(outs)

    devices = jax.devices()[:n_cores]
    mesh = Mesh(np.asarray(devices), ("core",))
    n_outs = len(out_avals)
    in_specs = (PartitionSpec("core"),) * (n_params + n_outs)
    out_specs = (PartitionSpec("core"),) * n_outs
    fn = jax.jit(shard_map(_body, mesh=mesh, in_specs=in_specs,
                           out_specs=out_specs, check_rep=False),
                 keep_unused=True)
    return fn, in_names, out_names, out_avals, zero_outs


def run(cfg: GatCfg, inputs, time_iters=0, repeats=1, in_maps=None):
    """Returns (out, best_exec_seconds or None)."""
    import time as _time
    import jax

    if in_maps is None:
        in_maps = shard_inputs(cfg, inputs)  # sets cfg.t_a / t_b
    nc = build_program(cfg, repeats=repeats)
    n_cores = cfg.n_cores
    fn, in_names, out_names, out_avals, zero_outs = _make_pjrt_fn(nc, n_cores)

    concat_in = [
        np.concatenate([np.asarray(in_maps[c][name]) for c in range(n_cores)], axis=0)
        for name in in_names
    ]
    concat_zero = [
        np.zeros((n_cores * z.shape[0], *z.shape[1:]), z.dtype) for z in zero_outs
    ]
    dev_in = [jax.device_put(a) for a in concat_in]
    dev_zero = [jax.device_put(a) for a in concat_zero]

    out_arrs = fn(*dev_in, *dev_zero)
    jax.block_until_ready(out_arrs)

    best = None
    if time_iters:
        times = []
        for _ in range(time_iters):
            t0 = _time.perf_counter()
            out_arrs2 = fn(*dev_in, *dev_zero)
            jax.block_until_ready(out_arrs2)
            times.append(_time.perf_counter() - t0)
        best = min(times)

    oi = out_names.index("out")
    full = np.asarray(out_arrs[oi]).reshape(n_cores, *out_avals[oi].shape)
    out = np.concatenate(list(full), axis=0)[: cfg.n]
    return out, best


def med_cfg():
    return GatCfg(
        n=8192,
        layers=(
            LayerCfg(256, 256, 8, 32, 320, True),
            LayerCfg(256, 256, 8, 32, 320, True),
            LayerCfg(256, 64, 1, 64, 128, False),
        ),
    )


# ---------------------------------------------------------------- entry point

def kernel(**inputs):
    """Full-input GAT kernel: shards across 8 NeuronCores internally,
    runs the Bass program via run_bass_kernel_spmd, returns [50000, 64] f32."""
    from concourse.bass_utils import run_bass_kernel_spmd

    cfg = real_cfg()
    in_maps = shard_inputs(cfg, inputs)  # sets cfg.t_a / t_b from edge_index
    nc = build_program(cfg)
    res = run_bass_kernel_spmd(nc, in_maps, core_ids=list(range(cfg.n_cores)))
    out = np.concatenate(
        [res.results[c]["out"] for c in range(cfg.n_cores)], axis=0)
    return out.astype(np.float32)



# revision 11
# speedup vs baseline: 1.6735x; 1.6735x over previous
"""3-layer GAT on Trainium2 — 8-core SPMD Bass kernel (bf16 datapath).

Node-partitioned (edge-cut) distribution:
- core c owns nodes [c*NPC, (c+1)*NPC) and all edges whose dst lands there
- per layer: each core computes its nodes' h_ext = in @ [W | W@As | W@Ad]
  (weight fusion done on host), publishes rows [h | als] bf16 to a
  replicated table via AllGather, then processes its incoming edges in
  windows of 128 dst nodes: dma_gather of source rows (negative-index
  tail padding so pad slots move no bytes; per-core valid counts come in
  through value_load registers), exp(leaky(als+ald)) on-chip, scatter-add
  via one-hot bf16 matmul into PSUM, normalize by the per-node sum.
- ald[dst] per edge via a second one-hot (dst-on-partitions) built with
  partition_broadcast + is_equal — no PE transposes in the edge phase.
"""
from dataclasses import dataclass

import numpy as np
import ml_dtypes

import concourse.bass as bass
import concourse.bacc as bacc
import concourse.mybir as mybir
import concourse.tile as tile

P = 128
N_CORES = 8
NEG_SLOPE = 0.2
BF = ml_dtypes.bfloat16


@dataclass
class LayerCfg:
    fin: int
    fout: int
    H: int
    C: int
    drow: int      # table row width in bf16 elems (multiple of 128)
    relu: bool

    @property
    def ex(self):
        return 2 * self.H

    @property
    def fw(self):
        return self.fout + self.ex

    @property
    def tw(self):
        # published table row payload: [h | als]
        return self.fout + self.H


@dataclass
class GatCfg:
    n: int
    n_cores: int = N_CORES
    layers: tuple = ()
    # filled by preprocessing
    t_a: int = 0
    t_b: int = 0
    ma: tuple = ()   # per-window valid A-slot count (max over cores)
    mb: tuple = ()   # per-window valid B-slot count (max over cores)

    @property
    def npc(self):
        return self.n // self.n_cores

    @property
    def n_win(self):
        return (self.npc + P - 1) // P

    @property
    def rows(self):
        return self.npc + 1

    @property
    def half(self):
        return self.rows * (self.n_cores // 2)

    @property
    def tbl(self):
        return self.rows * self.n_cores

    @property
    def t(self):
        return self.t_a + self.t_b


def real_cfg():
    return GatCfg(
        n=50000,
        layers=(
            LayerCfg(256, 256, 8, 32, 384, True),
            LayerCfg(256, 256, 8, 32, 384, True),
            LayerCfg(256, 64, 1, 64, 128, False),
        ),
    )


def mini_cfg():
    return GatCfg(
        n=1024,
        layers=(
            LayerCfg(256, 256, 8, 32, 384, True),
            LayerCfg(256, 256, 8, 32, 384, True),
            LayerCfg(256, 64, 1, 64, 128, False),
        ),
    )


def med_cfg():
    return GatCfg(
        n=8192,
        layers=(
            LayerCfg(256, 256, 8, 32, 384, True),
            LayerCfg(256, 256, 8, 32, 384, True),
            LayerCfg(256, 64, 1, 64, 128, False),
        ),
    )


# ---------------------------------------------------------------- host prep

def _wrap16(idx_list):
    """dma_gather index layout: idx i lives at [i % 16, i // 16]; replicate
    the 16-partition block 8x down to 128 partitions."""
    a = idx_list.reshape(-1, 16).T  # [16, len/16]
    return np.tile(a, (8, 1)).astype(np.int16)


def preprocess(cfg: GatCfg, edge_index):
    """Partition edges by dst core, group into 128-dst-node windows, split by
    src half (int16 index range), pad with -1 (skipped by the gather DMA).

    Returns (per_core_index_inputs, cfg with t_a/t_b set).
    """
    npc, n_win = cfg.npc, cfg.n_win
    src = np.concatenate([edge_index[0], np.arange(cfg.n, dtype=np.int64)])
    dst = np.concatenate([edge_index[1], np.arange(cfg.n, dtype=np.int64)])

    core = dst // npc
    # storage index of a node inside the 8*(npc+1)-row gathered table
    sidx = (src // npc) * cfg.rows + (src % npc)
    is_b = sidx >= cfg.half

    per_core = []
    counts_a = np.zeros((cfg.n_cores, n_win), dtype=np.int64)
    counts_b = np.zeros((cfg.n_cores, n_win), dtype=np.int64)
    for c in range(cfg.n_cores):
        sel = np.nonzero(core == c)[0]
        d_loc = (dst[sel] - c * npc).astype(np.int64)
        w = d_loc // P
        half_flag = is_b[sel].astype(np.int64)
        order = np.lexsort((d_loc, half_flag, w))
        sel = sel[order]
        d_loc = d_loc[order]
        w = w[order]
        half_flag = half_flag[order]
        rel = (sidx[sel] - half_flag * cfg.half).astype(np.int64)
        for wi in range(n_win):
            m = w == wi
            counts_a[c, wi] = int((m & (half_flag == 0)).sum())
            counts_b[c, wi] = int((m & (half_flag == 1)).sum())
        per_core.append((d_loc, w, half_flag, rel))

    assert counts_a.min() >= 1 and counts_b.min() >= 1
    t_a = max(int(np.ceil(counts_a.max() / P)), 1)
    t_b = max(int(np.ceil(counts_b.max() / P)), 1)
    cfg.t_a, cfg.t_b = t_a, t_b
    t = t_a + t_b
    # per-window valid slot counts, uniform across cores: shorter cores pad
    # with dummy-row gathers up to the max; slots beyond are -1 (no traffic)
    cfg.ma = tuple(int(v) for v in counts_a.max(axis=0))
    cfg.mb = tuple(int(v) for v in counts_b.max(axis=0))
    dummy = cfg.npc  # zero row at the end of each core's table slice

    idx_inputs = []
    for c in range(cfg.n_cores):
        d_loc, w, half_flag, rel = per_core[c]
        idx16 = np.zeros((P, n_win, t * 8), dtype=np.int16)
        dl16 = np.zeros((P, n_win, t), dtype=BF)
        dstb = np.zeros((n_win, t * P), dtype=BF)
        for wi in range(n_win):
            m = w == wi
            ra = rel[m & (half_flag == 0)]
            rb = rel[m & (half_flag == 1)]
            da = d_loc[m & (half_flag == 0)] % P
            db = d_loc[m & (half_flag == 1)] % P
            src_list = np.full(t * P, -1, dtype=np.int64)
            dl_list = np.full(t * P, -1, dtype=np.int64)
            src_list[: len(ra)] = ra
            src_list[len(ra) : cfg.ma[wi]] = dummy
            src_list[t_a * P : t_a * P + len(rb)] = rb
            src_list[t_a * P + len(rb) : t_a * P + cfg.mb[wi]] = dummy
            dl_list[: len(da)] = da
            dl_list[t_a * P : t_a * P + len(db)] = db
            idx16[:, wi, : t_a * 8] = _wrap16(src_list[: t_a * P])
            idx16[:, wi, t_a * 8 :] = _wrap16(src_list[t_a * P :])
            # dl in (p, j) layout: slot i -> [i % 128, i // 128]
            dl16[:, wi, :] = dl_list.reshape(t, P).T.astype(BF)
            dstb[wi] = dl_list.astype(BF)
        idx_inputs.append({"idx16": idx16, "dl16": dl16, "dstb": dstb})
    return idx_inputs


def shard_inputs(cfg: GatCfg, inputs):
    """Build the per-core input dicts for run_bass_kernel_spmd."""
    x = np.asarray(inputs["x"], dtype=np.float32)
    edge_index = np.asarray(inputs["edge_index"])
    idx_inputs = preprocess(cfg, edge_index)

    def blockdiag(a_s, a_d, fin_rows):
        H, C = a_s.shape
        A = np.zeros((fin_rows, 2 * H), dtype=np.float32)
        for h in range(H):
            A[h * C : (h + 1) * C, h] = a_s[h]
            A[h * C : (h + 1) * C, H + h] = a_d[h]
        return A

    weight_common = {}
    for li, nm in enumerate(["1", "2", "3"]):
        lc = cfg.layers[li]
        Wn = np.asarray(inputs[f"W{nm}"], dtype=np.float32)
        A = blockdiag(np.asarray(inputs[f"as{nm}"], np.float32),
                      np.asarray(inputs[f"ad{nm}"], np.float32), lc.fout)
        wext = np.concatenate([Wn, Wn @ A], axis=1)  # [fin, fw]
        weight_common[f"Wext{nm}"] = np.ascontiguousarray(wext).astype(BF)
        b = np.asarray(inputs[f"b{nm}"], np.float32).reshape(1, -1)
        weight_common[f"b{nm}"] = b.astype(BF) if lc.relu else b

    nw, fin = cfg.n_win, cfg.layers[0].fin
    kch = fin // P
    in_maps = []
    for c in range(cfg.n_cores):
        xp = np.zeros((nw * P, fin), dtype=np.float32)
        xp[: cfg.npc] = x[c * cfg.npc : (c + 1) * cfg.npc]
        # transposed layout: xT[p, q, w*128+n] = xp[w*128+n, q*128+p]
        xT = np.ascontiguousarray(
            xp.reshape(nw * P, kch, P).transpose(2, 1, 0)).astype(BF)
        m = {"xT": xT, **idx_inputs[c], **weight_common}
        in_maps.append(m)
    return in_maps


# ---------------------------------------------------------------- device code

def build_program(cfg: GatCfg, repeats: int = 1):
    nc = bacc.Bacc("TRN2", target_bir_lowering=False, debug=False,
                   num_devices=cfg.n_cores)
    n_win, t_a, t_b, t = cfg.n_win, cfg.t_a, cfg.t_b, cfg.t
    f32 = mybir.dt.float32
    bf = mybir.dt.bfloat16
    fin0 = cfg.layers[0].fin
    kch0 = fin0 // P

    # ---- I/O
    xT_in = nc.dram_tensor("xT", [P, kch0, n_win * P], bf, kind="ExternalInput")
    idx16 = nc.dram_tensor("idx16", [P, n_win, t * 8], mybir.dt.int16,
                           kind="ExternalInput")
    dl16 = nc.dram_tensor("dl16", [P, n_win, t], bf, kind="ExternalInput")
    dstb = nc.dram_tensor("dstb", [n_win, t * P], bf, kind="ExternalInput")
    wt_in = {}
    for li, nm in enumerate(["1", "2", "3"]):
        lc = cfg.layers[li]
        wt_in[f"Wext{nm}"] = nc.dram_tensor(f"Wext{nm}", [lc.fin, lc.fw], bf,
                                            kind="ExternalInput")
        wt_in[f"b{nm}"] = nc.dram_tensor(f"b{nm}", [1, lc.fout],
                                         bf if lc.relu else f32,
                                         kind="ExternalInput")
    out_t = nc.dram_tensor("out", [cfg.npc, cfg.layers[-1].fout], f32,
                           kind="ExternalOutput")

    # ---- internal DRAM (bf16 tables)
    hal12_loc = nc.dram_tensor("hal12_loc", [cfg.rows, 384], bf, kind="Internal")
    hal12_full = nc.dram_tensor("hal12_full", [cfg.tbl, 384], bf,
                                kind="Internal", addr_space="Shared")
    hal3_loc = nc.dram_tensor("hal3_loc", [cfg.rows, 128], bf, kind="Internal")
    hal3_full = nc.dram_tensor("hal3_full", [cfg.tbl, 128], bf,
                               kind="Internal", addr_space="Shared")

    iota_const = nc.inline_tensor(
        np.tile(np.arange(P), (P, 1)).astype(BF), name="iota_const")
    iotap_const = nc.inline_tensor(
        np.arange(P).reshape(P, 1).astype(BF), name="iotap_const")

    rg = [list(range(cfg.n_cores))]

    with tile.TileContext(nc) as tc:
        import contextlib
        with contextlib.ExitStack() as ctx:
            persist = ctx.enter_context(tc.tile_pool(name="persist", bufs=1))
            wpool = ctx.enter_context(tc.tile_pool(name="wts", bufs=1))
            sb = ctx.enter_context(tc.tile_pool(name="work", bufs=2))
            gp = ctx.enter_context(tc.tile_pool(name="gathp", bufs=3))
            sm = ctx.enter_context(tc.tile_pool(name="small", bufs=4))
            fr = ctx.enter_context(tc.tile_pool(name="front", bufs=2))
            ps = ctx.enter_context(tc.tile_pool(name="psum", bufs=2, space="PSUM"))
            psf = ctx.enter_context(tc.tile_pool(name="psumf", bufs=2, space="PSUM"))

            iota_sb = persist.tile([P, P], bf)
            nc.sync.dma_start(out=iota_sb[:], in_=iota_const[:])
            iota_p = persist.tile([P, 1], bf)
            nc.sync.dma_start(out=iota_p[:], in_=iotap_const[:])
            identity = persist.tile([P, P], bf)
            nc.vector.tensor_tensor(
                out=identity[:], in0=iota_sb[:],
                in1=iota_p[:].to_broadcast([P, P]),
                op=mybir.AluOpType.is_equal)

            xT_sb = persist.tile([P, kch0, n_win * P], bf)
            idx_all = persist.tile([P, n_win, t * 8], mybir.dt.int16)
            dl_all = persist.tile([P, n_win, t], bf)
            in_local = persist.tile([P, n_win, 256], bf)

            for _rep in range(repeats):
              nc.sync.dma_start(out=xT_sb[:], in_=xT_in[:])
              nc.scalar.dma_start(out=idx_all[:], in_=idx16[:])
              nc.scalar.dma_start(out=dl_all[:], in_=dl16[:])

              for li in range(len(cfg.layers)):
                  lc = cfg.layers[li]
                  nm = str(li + 1)
                  hal_loc = hal12_loc if lc.drow == 384 else hal3_loc
                  hal_full = hal12_full if lc.drow == 384 else hal3_full
                  kch = lc.fin // P  # input chunks (contraction)
                  gtag = "gath12" if lc.drow == 384 else "gath3"

                  # ---------- per-layer constants
                  w_ext = wpool.tile([P, kch, lc.fw], bf, tag=f"w_ext{nm}")
                  nc.sync.dma_start(
                      out=w_ext[:],
                      in_=wt_in[f"Wext{nm}"][:].rearrange("(q p) f -> p q f", p=P))
                  bdt = bf if lc.relu else f32
                  b_t = sm.tile([1, lc.fout], bdt, tag="b_t")
                  nc.sync.dma_start(out=b_t[:], in_=wt_in[f"b{nm}"][:])
                  b_bc = wpool.tile([P, lc.fout], bdt, tag=f"b_bc{nm}")
                  nc.gpsimd.partition_broadcast(b_bc[:], b_t[:1, :])
                  ald_all = wpool.tile([P, n_win * lc.H], bf, tag=f"ald{nm}")

                  # ---------- front phase: h_ext for own nodes -> hal_loc
                  for w in range(n_win):
                      if li == 0:
                          in_t = xT_sb[:, :, w * P : (w + 1) * P]
                      else:
                          in_tt = fr.tile([P, kch, P], bf, tag="in_t")
                          for q in range(kch):
                              tp = psf.tile([P, P], bf, tag="tp")
                              nc.tensor.transpose(
                                  tp[:], in_local[:, w, q * P : (q + 1) * P],
                                  identity[:])
                              nc.vector.tensor_copy(
                                  out=in_tt[:, q, :], in_=tp[:])
                          in_t = in_tt
                      h_ps = psf.tile([P, lc.fw], f32, tag="h_ps")
                      for q in range(kch):
                          nc.tensor.matmul(
                              h_ps[:],
                              lhsT=in_t[:, q, :],
                              rhs=w_ext[:, q, :],
                              start=(q == 0), stop=(q == kch - 1))
                      nc.scalar.copy(
                          out=ald_all[:, w * lc.H : (w + 1) * lc.H],
                          in_=h_ps[:, lc.fout + lc.H : lc.fw])
                      stage = fr.tile([P, lc.drow], bf, tag="stage")
                      nc.vector.tensor_copy(out=stage[:, : lc.tw],
                                            in_=h_ps[:, : lc.tw])
                      nc.vector.memset(stage[:, lc.tw :], 0.0)
                      rows = min(P, cfg.npc - w * P)
                      nc.sync.dma_start(
                          out=hal_loc[w * P : w * P + rows, :],
                          in_=stage[:rows, :])
                  # last (never-indexed) row must still be finite for the
                  # collective / finite checkers
                  dmy = sm.tile([1, lc.drow], bf, tag="dmy")
                  nc.vector.memset(dmy[:], 0.0)
                  nc.sync.dma_start(out=hal_loc[cfg.npc : cfg.npc + 1, :],
                                    in_=dmy[:])

                  nc.gpsimd.collective_compute(
                      "AllGather", mybir.AluOpType.bypass,
                      ins=[hal_loc[:]], outs=[hal_full[:]],
                      replica_groups=rg)

                  # ---------- edge phase
                  for w in range(n_win):
                      dstb_t = sm.tile([1, t * P], bf, tag="dstb_t")
                      nc.scalar.dma_start(out=dstb_t[:], in_=dstb[w : w + 1, :])
                      dstb_bc = sb.tile([P, t * P], bf, tag="dstb_bc")
                      nc.gpsimd.partition_broadcast(dstb_bc[:], dstb_t[:1, :])

                      gath = gp.tile([P, t, lc.drow], bf, tag=gtag)
                      # memset the never-gathered tail slots (static range)
                      for base, nt, mv in ((0, t_a, cfg.ma[w]),
                                           (t_a, t_b, cfg.mb[w])):
                          # memset whole tiles from the first incomplete one;
                          # the gather then rewrites the valid lanes
                          jp = mv // P
                          if jp < nt:
                              nc.vector.memset(
                                  gath[:, base + jp : base + nt, :], 0.0)
                      nc.gpsimd.dma_gather(
                          gath[:, :t_a, :], hal_full[: cfg.half, :],
                          idx_all[:, w, : t_a * 8],
                          num_idxs=t_a * P, num_idxs_reg=cfg.ma[w],
                          elem_size=lc.drow, single_packet=False)
                      nc.gpsimd.dma_gather(
                          gath[:, t_a:, :], hal_full[cfg.half :, :],
                          idx_all[:, w, t_a * 8 :],
                          num_idxs=t_b * P, num_idxs_reg=cfg.mb[w],
                          elem_size=lc.drow, single_packet=False)

                      # one-hot M^T (edge on partitions) for scatter
                      m_all = sb.tile([P, t * P], bf, tag="m_all")
                      nc.vector.tensor_tensor(
                          out=m_all[:].rearrange("p (t n) -> p t n", t=t),
                          in0=iota_sb[:].unsqueeze(1).to_broadcast([P, t, P]),
                          in1=dl_all[:, w, :].unsqueeze(2).to_broadcast([P, t, P]),
                          op=mybir.AluOpType.is_equal)
                      # one-hot M (dst on partitions) for ald broadcast
                      m2 = sb.tile([P, t * P], bf, tag="m2")
                      nc.vector.tensor_tensor(
                          out=m2[:],
                          in0=dstb_bc[:],
                          in1=iota_p[:].to_broadcast([P, t * P]),
                          op=mybir.AluOpType.is_equal)

                      # ald[dst] per edge via PE: M_j^T-free matmuls
                      aldps = psf.tile([P, t * lc.H], f32, tag="aldps")
                      for j in range(t):
                          nc.tensor.matmul(
                              aldps[:, j * lc.H : (j + 1) * lc.H],
                              lhsT=m2[:, j * P : (j + 1) * P],
                              rhs=ald_all[:, w * lc.H : (w + 1) * lc.H],
                              start=True, stop=True)

                      # e0 = als[src] + ald[dst]; exp(leaky(e0)) -> gath
                      e0 = sm.tile([P, t * lc.H], f32, tag="e0")
                      nc.vector.tensor_tensor(
                          out=e0[:].rearrange("p (t h) -> p t h", t=t),
                          in0=gath[:, :, lc.fout : lc.fout + lc.H],
                          in1=aldps[:].rearrange("p (t h) -> p t h", t=t),
                          op=mybir.AluOpType.add)
                      t1 = sm.tile([P, t * lc.H], f32, tag="t1")
                      nc.scalar.activation(t1[:], e0[:],
                                           mybir.ActivationFunctionType.Abs,
                                           scale=2.0 / 3.0)
                      nc.vector.tensor_tensor(out=e0[:], in0=e0[:], in1=t1[:],
                                              op=mybir.AluOpType.add)
                      nc.scalar.activation(
                          gath[:, :, lc.fout : lc.fout + lc.H],
                          e0[:].rearrange("p (t h) -> p t h", t=t),
                          mybir.ActivationFunctionType.Exp, scale=0.6)

                      # msg scale: h *= exp_e (broadcast over C)
                      h_view = gath[:, :, : lc.fout].rearrange(
                          "p t (h c) -> p t h c", h=lc.H)
                      expv = gath[:, :, lc.fout : lc.fout + lc.H].unsqueeze(
                          3).to_broadcast([P, t, lc.H, lc.C])
                      nc.vector.tensor_tensor(out=h_view, in0=h_view, in1=expv,
                                              op=mybir.AluOpType.mult)

                      # scatter-add via PE
                      acc = ps.tile([P, lc.tw], f32, tag="acc")
                      for j in range(t):
                          nc.tensor.matmul(
                              acc[:],
                              lhsT=m_all[:, j * P : (j + 1) * P],
                              rhs=gath[:, j, : lc.tw],
                              start=(j == 0), stop=(j == t - 1))

                      # normalize + bias (+relu) at node level
                      sden = sm.tile([P, lc.H], f32, tag="sden")
                      nc.vector.tensor_scalar(
                          out=sden[:], in0=acc[:, lc.fout : lc.tw],
                          scalar1=1e-12, scalar2=None, op0=mybir.AluOpType.max)
                      rec = sm.tile([P, lc.H], f32, tag="rec")
                      nc.vector.reciprocal(rec[:], sden[:])
                      recv = rec[:].unsqueeze(2).to_broadcast([P, lc.H, lc.C])
                      o_t = fr.tile([P, lc.fout], bf if lc.relu else f32,
                                    tag="o_t")
                      nc.vector.tensor_tensor(
                          out=o_t[:].rearrange("p (h c) -> p h c", h=lc.H),
                          in0=acc[:, : lc.fout].rearrange(
                              "p (h c) -> p h c", h=lc.H),
                          in1=recv, op=mybir.AluOpType.mult)
                      nc.vector.tensor_tensor(out=o_t[:], in0=o_t[:], in1=b_bc[:],
                                              op=mybir.AluOpType.add)
                      if lc.relu:
                          nc.vector.tensor_scalar(
                              out=in_local[:, w, : lc.fout], in0=o_t[:],
                              scalar1=0.0, scalar2=None, op0=mybir.AluOpType.max)
                      else:
                          rows = min(P, cfg.npc - w * P)
                          nc.sync.dma_start(
                              out=out_t[w * P : w * P + rows, :],
                              in_=o_t[:rows, :])

    nc.compile()
    return nc


# ---------------------------------------------------------------- runner

def _make_pjrt_fn(nc, n_cores):
    """Cached PJRT executable for nc (modeled on bass2jax.run_bass_via_pjrt,
    without output-buffer donation so it can be re-invoked for timing)."""
    import jax
    from jax.sharding import Mesh, PartitionSpec
    from jax.experimental.shard_map import shard_map
    from concourse import bass2jax, mybir as mb

    bass2jax.install_neuronx_cc_hook()
    partition_name = nc.partition_id_tensor.name if nc.partition_id_tensor else None
    in_names, out_names, out_avals, zero_outs = [], [], [], []
    for alloc in nc.m.functions[0].allocations:
        if not isinstance(alloc, mb.MemoryLocationSet):
            continue
        name = alloc.memorylocations[0].name
        if alloc.kind == "ExternalInput":
            if name != partition_name:
                in_names.append(name)
        elif alloc.kind == "ExternalOutput":
            out_names.append(name)
            shape = tuple(alloc.tensor_shape)
            dtype = mb.dt.np(alloc.dtype)
            out_avals.append(jax.core.ShapedArray(shape, dtype))
            zero_outs.append(np.zeros(shape, dtype))
    n_params = len(in_names)
    all_in_names = list(in_names) + list(out_names)
    if partition_name is not None:
        all_in_names.append(partition_name)

    def _body(*args):
        operands = list(args)
        if partition_name is not None:
            operands.append(bass2jax.partition_id_tensor())
        outs = bass2jax._bass_exec_p.bind(
            *operands,
            out_avals=tuple(out_avals),
            in_names=tuple(all_in_names),
            out_names=tuple(out_names),
            lowering_input_output_aliases=(),
            sim_require_finite=True,
            sim_require_nnan=True,
            nc=nc,
        )
        return tuple(outs)

    devices = jax.devices()[:n_cores]
    mesh = Mesh(np.asarray(devices), ("core",))
    n_outs = len(out_avals)
    in_specs = (PartitionSpec("core"),) * (n_params + n_outs)
    out_specs = (PartitionSpec("core"),) * n_outs
    fn = jax.jit(shard_map(_body, mesh=mesh, in_specs=in_specs,
                           out_specs=out_specs, check_rep=False),
                 keep_unused=True)
    return fn, in_names, out_names, out_avals, zero_outs


def run(cfg: GatCfg, inputs, time_iters=0, repeats=1, in_maps=None):
    """Returns (out, best_exec_seconds or None)."""
    import time as _time
    import jax

    if in_maps is None:
        in_maps = shard_inputs(cfg, inputs)  # sets cfg.t_a / t_b
    nc = build_program(cfg, repeats=repeats)
    n_cores = cfg.n_cores
    fn, in_names, out_names, out_avals, zero_outs = _make_pjrt_fn(nc, n_cores)

    concat_in = [
        np.concatenate([np.asarray(in_maps[c][name]) for c in range(n_cores)], axis=0)
        for name in in_names
    ]
    concat_zero = [
        np.zeros((n_cores * z.shape[0], *z.shape[1:]), z.dtype) for z in zero_outs
    ]
    dev_in = [jax.device_put(a) for a in concat_in]
    dev_zero = [jax.device_put(a) for a in concat_zero]

    out_arrs = fn(*dev_in, *dev_zero)
    jax.block_until_ready(out_arrs)

    best = None
    if time_iters:
        times = []
        for _ in range(time_iters):
            t0 = _time.perf_counter()
            out_arrs2 = fn(*dev_in, *dev_zero)
            jax.block_until_ready(out_arrs2)
            times.append(_time.perf_counter() - t0)
        best = min(times)

    oi = out_names.index("out")
    full = np.asarray(out_arrs[oi]).reshape(n_cores, *out_avals[oi].shape)
    out = np.concatenate(list(full), axis=0)[: cfg.n]
    return out, best


# ---------------------------------------------------------------- entry point

def kernel(**inputs):
    """Full-input GAT kernel: shards across 8 NeuronCores internally,
    runs the Bass program via run_bass_kernel_spmd, returns [50000, 64] f32."""
    from concourse.bass_utils import run_bass_kernel_spmd

    cfg = real_cfg()
    in_maps = shard_inputs(cfg, inputs)  # sets cfg.t_a / t_b from edge_index
    nc = build_program(cfg)
    res = run_bass_kernel_spmd(nc, in_maps, core_ids=list(range(cfg.n_cores)))
    out = np.concatenate(
        [res.results[c]["out"] for c in range(cfg.n_cores)], axis=0)
    return out.astype(np.float32)
